# revision 1
# baseline (speedup 1.0000x reference)
"""Trainium2 Bass kernel for MFVIConstituency mean-field iterations.

Per batch b (one NeuronCore each, 8 total):
    q = s_con;  repeat 3x:  q[i,j] = s_con[i,j] + sum_k sig(q)[j,k] * sb[i,j,k]
    out = sigmoid(q)
where sb = s_bin * mask2o, mask2o[i,j,k] = mask[i,j] & (i!=k) & (j!=k).

Formulation: the contraction is a batch of 192 per-j matvecs
    q[:, j] = SB_j @ sig(q)[j, :],   SB_j = sb[:, j, :]  (192x192)
mapped onto the TensorEngine: for each output column j the stationary
operand is sb[k, i; j] (k-tiles 128+64, i-tiles 128+64) and the moving
operand is the single column sig(q)^T[:, j]; 4 matmuls accumulate
q[:, j] in PSUM (fp32).  s_con enters first through identity-stationary
matmuls (start=True sets has_written for the whole tile; a per-column
start would clear has_written BANK-wide and break accumulation).  The
two i-halves of q share one PSUM bank ([128, 384]: rows 0:128 at cols
0:192, rows 128:192 at cols 192:384 on partitions 0:64) so one
activation instruction with a [p, 2, c] access pattern sigmoids both.

s_bin lives in SBUF as fp16.  The DMA cost model charges free-dim bytes
per partition (partition count is free), so everything is packed into
128 partitions: the 64-row k-tile-2 blocks ride the upper partition
half (two j-blocks sharing 128 partitions), and the whole 14.2MB cache
is striped over the three DMA queues (SP / Activation / GpSimd) in
8-column j-blocks, round-robin, so the three transfers overlap and
columns arrive in j order; iteration-1 matmuls stream right behind.
Because lhsT and rhs must share a base partition, sig(q)^T k-rows
128:192 are kept duplicated on partitions 0:64 AND 64:128 (the PE
transposes write each block twice; one DVE copy moves both).

Boundary between iterations builds X = sigmoid(q) (fp16) and R = X^T,
split by column halves so the left half (q cols 0:128) runs while the
right-half matmuls / DMA are still in flight: ACT sigmoid (PSUM->SBUF,
the first one also absorbs the act-table load under the DMA phase) ->
PE transpose blocks -> DVE copies over adjacent column ranges.  Output
leaves in natural [i, j] layout.
"""

import numpy as np

S = 192
B = 8
P = 128
K2 = 64          # k-tile-2 rows (k 128:192), also lower half of i
BJ = 8           # j per block
NB = S // BJ     # 24 blocks, striped round-robin over 3 queues
BW = BJ * S      # 1536 elements per (block, k-tile)
SEG = 3 * BW     # 4608 elements per block-pair segment
QW = 4 * SEG     # 18432 elements per queue tensor

_CACHE = {}


def _wslices(j):
    """j -> (queue, w1 col base, w2 col base, w2 partition range)."""
    b, jj = divmod(j, BJ)
    q, m = b % 3, b // 3
    p, which = divmod(m, 2)
    c1 = p * SEG + which * BW + jj * S
    c2 = p * SEG + 2 * BW + jj * S
    pr = (0, K2) if which == 0 else (K2, P)
    return q, c1, c2, pr


def _rcols(j):
    """r-tile columns for (R1 col j, R2 col j) in the packed layout
    [R1 cols 0:128 | R2 cols 0:128 | R1 cols 128:192 | R2 cols 128:192]."""
    return (j, P + j) if j < P else (P + j, S + j)


def _build_program():
    import concourse.tile as tile
    from concourse import mybir, bacc
    from contextlib import ExitStack

    f32, f16 = mybir.dt.float32, mybir.dt.float16
    Sig = mybir.ActivationFunctionType.Sigmoid

    nc = bacc.Bacc("TRN2", target_bir_lowering=False, debug=False, num_devices=B)
    wq_d = [nc.dram_tensor(f"wq{q}", [P, QW], f16, kind="ExternalInput")
            for q in range(3)]
    ident_d = nc.dram_tensor("ident", [P, P], f16, kind="ExternalInput")
    rr0_d = nc.dram_tensor("rr0", [P, 2 * S], f16, kind="ExternalInput")
    scon_d = nc.dram_tensor("scon", [P, 2 * S], f16, kind="ExternalInput")
    q_d = nc.dram_tensor("q_out", [S, S], f32, kind="ExternalOutput")

    def lrv(ap, lo, hi):
        """[p, 384] tile view -> [p, 2, hi-lo] AP over cols {lo:hi, 192+lo:192+hi}."""
        return ap.rearrange("p (s c) -> p s c", c=S)[:, :, lo:hi]

    with tile.TileContext(nc) as tc, ExitStack() as ctx:
        w_p = ctx.enter_context(tc.tile_pool(name="w", bufs=1))
        c_p = ctx.enter_context(tc.tile_pool(name="const", bufs=1))
        r_p = ctx.enter_context(tc.tile_pool(name="r", bufs=2))
        x_p = ctx.enter_context(tc.tile_pool(name="x", bufs=2))
        o_p = ctx.enter_context(tc.tile_pool(name="o", bufs=1))
        qq_p = ctx.enter_context(tc.tile_pool(name="qq", bufs=2, space="PSUM"))
        t_p = ctx.enter_context(tc.tile_pool(name="t", bufs=1, space="PSUM"))

        # one merged const DMA per queue, at the stream head
        ident_t = c_p.tile([P, P], f16, tag="ident")
        nc.sync.dma_start(ident_t[:], ident_d.ap())
        rr_t = r_p.tile([P, 2 * S], f16, tag="rr")
        nc.scalar.dma_start(rr_t[:], rr0_d.ap())
        scon_t = c_p.tile([P, 2 * S], f16, tag="scon")
        nc.gpsimd.dma_start(scon_t[:], scon_d.ap())

        # s_bin cache: 3 striped queue tensors, 4 segment chunks each
        wt = [w_p.tile([P, QW], f16, tag=f"wq{q}", name=f"wq{q}") for q in range(3)]
        queues = [nc.sync, nc.scalar, nc.gpsimd]
        for p in range(4):
            sl = slice(p * SEG, (p + 1) * SEG)
            for q in range(3):
                queues[q].dma_start(wt[q][:, sl], wq_d[q].ap()[:, sl])

        def col_matmuls(qq, rr_t, j0, j1):
            for j in range(j0, j1):
                q, c1, c2, (p0, p1) = _wslices(j)
                r1c, r2c = _rcols(j)
                last = j == S - 1
                t = wt[q]
                rj1 = rr_t[:, r1c:r1c + 1]
                rj2 = rr_t[p0:p1, r2c:r2c + 1]
                nc.tensor.matmul(qq[:, j:j + 1], t[:, c1:c1 + P], rj1,
                                 start=False, stop=False, skip_group_check=True)
                nc.tensor.matmul(qq[:, j:j + 1], t[p0:p1, c2:c2 + P], rj2,
                                 start=False, stop=False, skip_group_check=True)
                nc.tensor.matmul(qq[0:K2, S + j:S + j + 1],
                                 t[:, c1 + P:c1 + S], rj1,
                                 start=False, stop=last, skip_group_check=True)
                nc.tensor.matmul(qq[0:K2, S + j:S + j + 1],
                                 t[p0:p1, c2 + P:c2 + S], rj2,
                                 start=False, stop=last, skip_group_check=True)

        # tt columns: [t1 0:128 | t2 128:256 | t3 256:320 | t4 320:384]
        CT2, CT3, CT4 = P, 2 * P, 2 * P + K2

        def boundary_left(qq, xx, tt, rrn):
            nc.scalar.activation(lrv(xx[:], 0, P), lrv(qq[:], 0, P), Sig)
            nc.tensor.transpose(tt[:, 0:P], xx[:, 0:P], ident_t[:])
            nc.tensor.transpose(tt[:, CT3:CT3 + K2], xx[0:K2, S:S + P],
                                ident_t[0:K2, 0:K2])
            nc.vector.tensor_copy(rrn[:, 0:P], tt[:, 0:P])

        def boundary_right(qq, xx, tt, rrn):
            nc.scalar.activation(lrv(xx[:], P, S), lrv(qq[:], P, S), Sig)
            nc.tensor.transpose(tt[0:K2, CT2:CT2 + P], xx[:, P:S], ident_t[:])
            nc.tensor.transpose(tt[K2:P, CT2:CT2 + P], xx[:, P:S], ident_t[:])
            nc.tensor.transpose(tt[0:K2, CT4:CT4 + K2], xx[0:K2, S + P:2 * S],
                                ident_t[0:K2, 0:K2])
            nc.tensor.transpose(tt[K2:P, CT4:CT4 + K2], xx[0:K2, S + P:2 * S],
                                ident_t[0:K2, 0:K2])
            nc.vector.tensor_copy(rrn[:, P:2 * P], tt[:, CT2:CT2 + P])
            nc.vector.tensor_copy(rrn[:, 2 * P:2 * S], tt[:, CT3:3 * P])

        for it in range(3):
            qq = qq_p.tile([P, 2 * S], f32, tag="qq")
            # q = s_con first (identity stationary: out[m,c] = rhs[m,c]).
            # The second matmul spans all 128 partitions (zeros on 64:128)
            # so the combined sigmoid below reads only written PSUM.
            nc.tensor.matmul(qq[:, 0:S], ident_t[:], scon_t[:, 0:S],
                             start=True, stop=False, skip_group_check=True)
            nc.tensor.matmul(qq[:, S:2 * S], ident_t[0:K2, :],
                             scon_t[0:K2, S:2 * S],
                             start=False, stop=False, skip_group_check=True)
            if it < 2:
                xx = x_p.tile([P, 2 * S], f16, tag="xx")
                tt = t_p.tile([P, 3 * P], f16, tag="tt")
                rrn = r_p.tile([P, 2 * S], f16, tag="rr")
                if it == 0:
                    # DMA-bound: left-half boundary hides under the load
                    col_matmuls(qq, rr_t, 0, P)
                    boundary_left(qq, xx, tt, rrn)
                    col_matmuls(qq, rr_t, P, S)
                else:
                    col_matmuls(qq, rr_t, 0, S)
                    boundary_left(qq, xx, tt, rrn)
                boundary_right(qq, xx, tt, rrn)
                rr_t = rrn
            else:
                oo = o_p.tile([P, 2 * S], f32, tag="oo")
                col_matmuls(qq, rr_t, 0, P)
                nc.scalar.activation(lrv(oo[:], 0, P), lrv(qq[:], 0, P), Sig)
                nc.sync.dma_start(q_d.ap()[0:P, 0:P], oo[:, 0:P])
                nc.gpsimd.dma_start(q_d.ap()[P:S, 0:P], oo[0:K2, S:S + P])
                col_matmuls(qq, rr_t, P, S)
                nc.scalar.activation(lrv(oo[:], P, S), lrv(qq[:], P, S), Sig)
                nc.sync.dma_start(q_d.ap()[0:P, P:S], oo[:, P:S])
                nc.gpsimd.dma_start(q_d.ap()[P:S, P:S], oo[0:K2, S + P:2 * S])
    nc.compile()
    return nc


def _get_program():
    if "nc" not in _CACHE:
        _CACHE["nc"] = _build_program()
    return _CACHE["nc"]


def _prep_core_inputs(s_con_b, sbm16_b, ident):
    """Per-batch input dict. sbm16_b: masked s_bin, fp16, [i, j, k]."""
    kt = np.ascontiguousarray(sbm16_b.transpose(2, 1, 0))   # [k, j, i]
    w1 = kt[0:P].reshape(P, NB, BW)                          # k 0:128
    w2 = kt[P:S].reshape(K2, NB, BW)                         # k 128:192
    out = {"ident": ident}
    for q in range(3):
        bs = [q + 3 * m for m in range(NB // 3)]
        segs = []
        for p in range(4):
            b0, b1 = bs[2 * p], bs[2 * p + 1]
            segs.append(np.concatenate(
                [w1[:, b0], w1[:, b1],
                 np.concatenate([w2[:, b0], w2[:, b1]], axis=0)], axis=1))
        out[f"wq{q}"] = np.ascontiguousarray(np.concatenate(segs, axis=1))
    sig0 = (1.0 / (1.0 + np.exp(-s_con_b.astype(np.float64)))).astype(np.float16)
    r1 = sig0[:, 0:P].T                                      # [k 0:128, j]
    r2 = sig0[:, P:S].T                                      # [k 128:192, j]
    r2d = np.concatenate([r2, r2], axis=0)                   # duplicated halves
    out["rr0"] = np.ascontiguousarray(np.concatenate(
        [r1[:, 0:P], r2d[:, 0:P], r1[:, P:S], r2d[:, P:S]], axis=1))
    sc16 = s_con_b.astype(np.float16)
    scon = np.zeros((P, 2 * S), dtype=np.float16)
    scon[:, 0:S] = sc16[0:P]
    scon[0:K2, S:2 * S] = sc16[P:S]
    out["scon"] = scon
    return out


def kernel(s_con, s_bin, mask):
    from concourse.bass_utils import run_bass_kernel_spmd

    s_con = np.asarray(s_con, dtype=np.float32)
    s_bin = np.asarray(s_bin, dtype=np.float32)
    mask = np.asarray(mask)

    idx = np.arange(S)
    ne = idx[:, None] != idx[None, :]                       # [a, k]
    m2 = ne[:, None, :] & ne[None, :, :]                    # [i, j, k]
    full_mask = mask[:, :, :, None] & m2[None]              # [B, i, j, k]
    sbm16 = (s_bin * full_mask).astype(np.float16)

    ident = np.eye(P, dtype=np.float16)
    nc = _get_program()
    in_maps = [_prep_core_inputs(s_con[b], sbm16[b], ident) for b in range(B)]
    res = run_bass_kernel_spmd(nc, in_maps, list(range(B)))
    out = np.stack([res.results[b]["q_out"] for b in range(B)], 0)
    return np.ascontiguousarray(out.astype(np.float32))

